# revision 75
# baseline (speedup 1.0000x reference)
"""Trainium2 Bass kernel for nn_Attention_33921651703853 (sparse_attention).

Data-parallel over batch: B=256 -> 32 batches on each of 8 NeuronCores.
All weights replicated; no collectives.

Design (v4 — software-pipelined, head-batched, uniform-shape matmuls):
  - Everything transposed on device (no on-device transposes):
    xT tiles [d-part, token-free] bf16, host-pretiled; scores computed as
    sT[j, i]; AV gives outT[d, i]; final projection consumes outT directly.
  - EVERY matmul is padded to a K=128 x M=128 PE-array shape: switching the
    array tile config between matmuls flushes the PE pipeline (measured
    3.2x slowdown on alternating-shape streams). Token/row tails (36 of
    164) read past their logical end into small zero/garbage extensions;
    K-padding is exact because probs1/v/raq/h1 carry zeroed pad rows.
  - Attention is processed per batch with all 8 heads batched:
    scores for head-groups (3,3,2) land in shared PSUM tiles and are
    evacuated in one activation per group (mask folded as per-partition
    bias). The keypoint-block MLP runs once per batch over [100, 8*100]
    with W1/W2 zero-padded to 128x128, its epilogues are single DVE ops,
    xian is broadcast across heads with a stride-0 AP. exp runs as one
    activation over [128, 8*164]; the j-tail goes psum->exp directly with
    the mask bias folded into Exp (no SBUF staging).
  - Softmax denominators: probs0+probs1 are pre-added on DVE into the dead
    sT0 tile reinterpreted as bf16 (partition sums are row-order agnostic),
    so one ones-matmul per 512-col chunk computes all 8 heads' denominators
    broadcast across partitions. Reciprocal on DVE; AV results are
    normalized during the PSUM->SBUF evacuation multiply per head group.
  - Batches are software-pipelined: step i emits phase A(i) (scores+MLP+exp)
    interleaved with phase B(i-1) (denom+AV) of the previous batch, so no
    engine waits on a freshly produced dependency.
  - Final projection is packed across batch PAIRS: outT pair tiles
    [128, 8h*328] give i-tiles (128,128,72+pad) instead of (128,36)x2 ->
    25% fewer streamed PE columns. y rows are contiguous global tokens.
  - QKV weights are stored ct-major (wqk) and streamed in 16 chunks on the
    gpsimd DMA queue (xt on the sync queue in parallel); dummy warmup
    matmuls on onesm ramp the PE p-state while the first DMAs land.
    Projection/final chains of the next group are zipped into the current
    group's attention stream with deadline pacing, and the last pairs'
    final chains are held back to cover the pipeline drain.
  Result: TensorE ~97% busy at ~0.44 ns/col (HFU ~0.90); 1376us -> 830us.
"""

import os
from collections import deque
import numpy as np
import ml_dtypes

import concourse.bass as bass
import concourse.bacc as bacc
import concourse.mybir as mybir
import concourse.tile as tile
from concourse import bass_utils

BF16 = mybir.dt.bfloat16
F32 = mybir.dt.float32
AF = mybir.ActivationFunctionType
ALU = mybir.AluOpType
nbf16 = ml_dtypes.bfloat16

B, N, D, H, NK, DH = 256, 164, 1024, 8, 100, 128
NCORES = 8
BPC = B // NCORES          # 32 batches per core
GB = 4                     # batches per group
NG = BPC // GB             # 8 groups
XCOLS = GB * N             # 656 free cols per d-tile in xT sbuf
PW = 2 * N                 # 328 pair width (final projection packing)
N1 = N - 128               # 36 tail rows
MASK_NEG = -1.0e30
HGRP = [(0, 3), (3, 3), (6, 2)]   # head groups for score/AV psum batching
# Every matmul is padded to a K=128 x M=128 PE-array shape: switching the
# array tile config between matmuls flushes the PE pipeline (measured 3.2x
# slowdown on alternating-shape streams). Stationary operands read up to 92
# columns past their logical end, so the staging tiles carry a small
# zero-initialized extension.
EXT = 92
QKT_W = 16 * XCOLS + EXT
XT_W = 8 * XCOLS + EXT
OUTT_W = H * PW + 56

_CACHE = {}
LAST_EXEC_NS = None


def _install_profile_hook():
    """Make run_bass_kernel_spmd(trace=True) work under axon in this image."""
    import sys as _sys
    import types as _types
    try:
        import antenv  # noqa: F401
        try:
            from antenv.axon_hooks import get_axon_ntff_profile_hook  # noqa: F401
        except ImportError:
            from trn_agent_boot.trn_boot import _ntff_profile_via_ctypes
            hook = _ntff_profile_via_ctypes("/opt/axon/libaxon_pjrt.so")
            mod = _types.ModuleType("antenv.axon_hooks")
            mod._hook = hook
            mod.set_axon_ntff_profile_hook = lambda h: setattr(mod, "_hook", h)
            mod.get_axon_ntff_profile_hook = lambda: mod._hook
            _sys.modules["antenv.axon_hooks"] = mod
            antenv.axon_hooks = mod
        if not getattr(bass_utils, "_upload_patched", False):
            _orig_upload = bass_utils.upload_artifacts

            def _safe_upload(tmpdir):
                try:
                    return _orig_upload(tmpdir)
                except Exception:
                    return tmpdir

            bass_utils.upload_artifacts = _safe_upload
            bass_utils._upload_patched = True
        return True
    except Exception as e:  # pragma: no cover
        print(f"profile hook install failed: {type(e).__name__}: {e}")
        return False


def _build_nc():
    nc = bacc.Bacc("TRN2", target_bir_lowering=False, debug=False)

    # ---- DRAM parameters (per-core shapes) ----
    d_xt = nc.dram_tensor("xt", [NG, 128, 8 * XCOLS], BF16, kind="ExternalInput")
    d_wqk = nc.dram_tensor("wqk", [128, 16 * 1024], BF16, kind="ExternalInput")
    d_wv = nc.dram_tensor("wv", [128, 8 * 1024], BF16, kind="ExternalInput")
    d_wout = nc.dram_tensor("wout", [128, 8 * D], BF16, kind="ExternalInput")
    d_w1 = nc.dram_tensor("w1", [NK, 50], BF16, kind="ExternalInput")
    d_w2 = nc.dram_tensor("w2", [50, NK], BF16, kind="ExternalInput")
    d_b1 = nc.dram_tensor("b1c", [50, 1], F32, kind="ExternalInput")
    d_b2 = nc.dram_tensor("b2c", [NK, 1], F32, kind="ExternalInput")
    d_boutb = nc.dram_tensor("boutb", [128, D], BF16, kind="ExternalInput")
    d_mbt0 = nc.dram_tensor("mbt0", [128, BPC], F32, kind="ExternalInput")
    d_mbt1 = nc.dram_tensor("mbt1", [N1, BPC], F32, kind="ExternalInput")
    d_xiant = nc.dram_tensor("xiant", [NG, NK, GB * NK], BF16, kind="ExternalInput")
    d_y = nc.dram_tensor("y", [BPC * N, D], BF16, kind="ExternalOutput")

    xt_ap = d_xt.ap()
    y_ap = d_y.ap()
    xiant_ap = d_xiant.ap()

    with tile.TileContext(nc) as tc:
        with (
            tc.tile_pool(name="const", bufs=1) as cpool,
            tc.tile_pool(name="xt", bufs=2) as xt_pool,
            tc.tile_pool(name="xian", bufs=2) as xian_pool,
            tc.tile_pool(name="qk", bufs=2) as qk_pool,
            tc.tile_pool(name="st0", bufs=2) as st0_pool,
            tc.tile_pool(name="pr0", bufs=1) as pr0_pool,
            tc.tile_pool(name="mlp", bufs=1) as mlp_pool,
            tc.tile_pool(name="rbcp", bufs=1) as rbc_pool,
            tc.tile_pool(name="outT", bufs=2) as outT_pool,
            tc.tile_pool(name="ysb", bufs=3) as y_pool,
            tc.tile_pool(name="pproj", bufs=2, space="PSUM") as pp,
            tc.tile_pool(name="psc", bufs=3, space="PSUM") as psc,
            tc.tile_pool(name="pmx", bufs=3, space="PSUM") as pm,
        ):
            # ---- constant tiles ----
            wqk_sb = cpool.tile([128, 16 * 1024], BF16, tag="wqk")
            wv_sb = cpool.tile([128, 8 * 1024], BF16, tag="wv")
            wout_sb = cpool.tile([128, 8 * D], BF16, tag="wout")
            w1_sb = cpool.tile([128, 128], BF16, tag="w1")
            w2_sb = cpool.tile([128, 128], BF16, tag="w2")
            b1_sb = cpool.tile([50, 1], F32, tag="b1")
            b2_sb = cpool.tile([NK, 1], F32, tag="b2")
            boutb_sb = cpool.tile([128, D], BF16, tag="boutb")
            mbt0_sb = cpool.tile([128, BPC], F32, tag="mbt0")
            mbt1_sb = cpool.tile([N1, BPC], F32, tag="mbt1")
            onesm_sb = cpool.tile([128, 128], BF16, tag="onesm")
            nc.vector.memset(onesm_sb[:], 1.0)
            # persistent double buffers with zero-padded K rows (so padded
            # K=128 matmuls contribute exactly zero from rows past the data)
            # memset bases must be 32-aligned; rows below the logical pad line
            # are rewritten by the regular per-batch writes before any read
            def zero_rows(t):
                # partition spans >32 must start at 0 or 64
                nc.vector.memset(t[32:64, :], 0.0)
                nc.vector.memset(t[64:128, :], 0.0)

            v_bufs = []
            for vi in range(2):
                vt = cpool.tile([128, GB * 2 * D], BF16, tag=f"v{vi}", name=f"v{vi}")
                zero_rows(vt)
                v_bufs.append(vt)
            p1_bufs = []
            for vi in range(2):
                p1 = cpool.tile([128, H * N], BF16, tag=f"p1{vi}", name=f"p1{vi}")
                zero_rows(p1)
                p1_bufs.append(p1)
            raq_bufs = []
            for vi in range(2):
                rq = cpool.tile([128, H * NK], BF16, tag=f"rq{vi}", name=f"rq{vi}")
                nc.vector.memset(rq[96:128, :], 0.0)
                raq_bufs.append(rq)
            h1_bufs = []
            for vi in range(2):
                hh = cpool.tile([128, H * NK], BF16, tag=f"h{vi}", name=f"h{vi}")
                zero_rows(hh)
                h1_bufs.append(hh)
            nc.vector.memset(w1_sb[:], 0.0)
            nc.vector.memset(w2_sb[:], 0.0)

            def load_consts_early():
                # small tiles needed by the first attention batch
                nc.sync.dma_start(w1_sb[:NK, :50], d_w1.ap()[:, :])
                nc.sync.dma_start(w2_sb[:50, :NK], d_w2.ap()[:, :])
                nc.sync.dma_start(b1_sb[:], d_b1.ap()[:, :])
                nc.sync.dma_start(b2_sb[:], d_b2.ap()[:, :])
                nc.sync.dma_start(mbt0_sb[:], d_mbt0.ap()[:, :])
                nc.sync.dma_start(mbt1_sb[:], d_mbt1.ap()[:, :])

            def load_consts_late():
                nc.gpsimd.dma_start(wv_sb[:, :4096], d_wv.ap()[:, :4096])
                nc.gpsimd.dma_start(wv_sb[:, 4096:], d_wv.ap()[:, 4096:])
                nc.gpsimd.dma_start(wout_sb[:, :4096], d_wout.ap()[:, :4096])
                nc.gpsimd.dma_start(wout_sb[:, 4096:], d_wout.ap()[:, 4096:])
                nc.gpsimd.dma_start(boutb_sb[:], d_boutb.ap()[:, :])

            # ---- projection chains ----
            group_tiles = {}

            def start_group(g):
                """Issue group g's input DMAs, allocate tiles, return the list
                of projection-chain closures (qk then v)."""
                xt_sb = xt_pool.tile([128, XT_W], BF16, tag="xt")
                nc.sync.dma_start(xt_sb[:, :8 * XCOLS], xt_ap[g, :, :])
                nc.vector.memset(xt_sb[:, 8 * XCOLS:], 0.0)
                xian_sb = xian_pool.tile([NK, GB * NK], BF16, tag="xian")
                nc.sync.dma_start(xian_sb[:], xiant_ap[g, :, :])
                qkT = qk_pool.tile([128, QKT_W], BF16, tag="qkT")
                nc.vector.memset(qkT[:, 16 * XCOLS:], 0.0)
                v_sb = v_bufs[g % 2]
                group_tiles[g] = (xt_sb, qkT, v_sb, xian_sb)

                def qk_chain(ct, c0, cw):
                    pt = pp.tile([128, 512], F32, tag="proj")
                    for dt in range(8):
                        nc.tensor.matmul(
                            pt[:, :cw],
                            wqk_sb[:, ct * 1024 + dt * 128:ct * 1024 + dt * 128 + 128],
                            xt_sb[:, dt * XCOLS + c0:dt * XCOLS + c0 + cw],
                            start=(dt == 0), stop=(dt == 7),
                        )
                        if dt < 7:
                            yield
                    nc.scalar.activation(
                        qkT[:, ct * XCOLS + c0:ct * XCOLS + c0 + cw],
                        pt[:, :cw], AF.Copy,
                    )

                def v_chain(b, tt, ch):
                    # stationary always 128 tokens; the tt=1 tile reads 92
                    # cols past the batch tail (next batch / zero extension),
                    # producing dead psum rows 36:128 that are never evacuated
                    p0, pw = (0, 128) if tt == 0 else (128, N1)
                    pt = pp.tile([128, 512], F32, tag="proj")
                    for dt in range(8):
                        nc.tensor.matmul(
                            pt[:, :],
                            xt_sb[:, dt * XCOLS + b * N + p0:dt * XCOLS + b * N + p0 + 128],
                            wv_sb[:, dt * 1024 + ch * 512:dt * 1024 + ch * 512 + 512],
                            start=(dt == 0), stop=(dt == 7),
                        )
                        if dt < 7:
                            yield
                    nc.vector.tensor_copy(
                        v_sb[:pw, (b * 2 + tt) * D + ch * 512:(b * 2 + tt) * D + ch * 512 + 512],
                        pt[:pw, :],
                    )

                if g < NG - 1:
                    chains = []
                    for ct in range(16):
                        for c0, cw in ((0, 512), (512, XCOLS - 512)):
                            chains.append(qk_chain(ct, c0, cw))
                    for b in range(GB):
                        for tt in range(2):
                            for ch in range(2):
                                chains.append(v_chain(b, tt, ch))
                    return chains, None
                # last group: split per pair; pair chains must finish before
                # that pair's attention
                pairs = []
                for p in range(2):
                    L = []
                    for ct in range(16):
                        L.append(qk_chain(ct, p * PW, PW))
                    for b in (2 * p, 2 * p + 1):
                        for tt in range(2):
                            for ch in range(2):
                                L.append(v_chain(b, tt, ch))
                    pairs.append(L)
                return pairs[0], pairs[1]

            # ---- final projection (packed per batch pair) ----
            def final_chain(outT_p, p, it):
                i0 = it * 128
                iw = 128 if it < 2 else PW - 256
                for ch in range(2):
                    y_sb = y_pool.tile([128, 512], BF16, tag="y")
                    yp = pp.tile([128, 512], F32, tag="proj")
                    for h2 in range(H):
                        nc.tensor.matmul(
                            yp[:, :],
                            outT_p[:, h2 * PW + i0:h2 * PW + i0 + 128],
                            wout_sb[:, h2 * D + ch * 512:h2 * D + ch * 512 + 512],
                            start=(h2 == 0), stop=(h2 == 7),
                        )
                        if h2 < 7:
                            yield
                    nc.vector.tensor_add(
                        y_sb[:iw, :],
                        yp[:iw, :],
                        boutb_sb[:iw, ch * 512:ch * 512 + 512],
                    )
                    nc.sync.dma_start(
                        y_ap[p * PW + i0:p * PW + i0 + iw, ch * 512:ch * 512 + 512],
                        y_sb[:iw, :])

            # ---- work queues & pacing (generator granularity: 1 unit = 1 MM) ----
            work = deque()     # projection chain generators (window deadlines)
            finals = deque()   # final-chain generators (soft deadlines)
            cur = {"w": None, "f": None}

            def _run(gen):
                for _ in gen:
                    pass

            def pump(n):
                # n counts whole chains; chains have window deadlines
                while n > 0 and (work or finals):
                    q = work if work else finals
                    _run(q.popleft())
                    n -= 1

            def pump_finals(n):
                while n > 0 and finals:
                    _run(finals.popleft())
                    n -= 1

            def pump_work_all():
                while work:
                    _run(work.popleft())

            def work_units():
                return len(work)

            # ---- attention phase A: scores + MLP + exp for batch gi ----
            state = {}
            pair_outT = {}

            def emit_A(gi, pumps):
                g, b = divmod(gi, GB)
                xt_sb, qkT, v_sb, xian_sb = group_tiles[g]
                half = b & 1
                if half == 0:
                    outT_p = outT_pool.tile([128, OUTT_W], BF16, tag="outT")
                    nc.vector.memset(outT_p[:, H * PW:], 0.0)
                    pair_outT[gi // 2] = outT_p
                else:
                    outT_p = pair_outT[gi // 2]
                sT0 = st0_pool.tile([128, H * N], F32, tag="sT0")
                probs0 = pr0_pool.tile([128, H * N], BF16, tag="p0")
                probs1 = p1_bufs[gi % 2]
                # scores per head group; all matmuls K=128 x M=128 (the k-tail
                # stationary reads 92 cols past the batch: dead psum rows)
                for h0, gw in HGRP:
                    sp0 = psc.tile([128, 492], F32, tag="sc")
                    sp1 = pm.tile([128, 512], F32, tag="pm")
                    for k in range(gw):
                        h = h0 + k
                        qof = h * XCOLS + b * N
                        kof = (8 + h) * XCOLS + b * N
                        nc.tensor.matmul(sp0[:, k * N:k * N + N],
                                         qkT[:, kof:kof + 128],
                                         qkT[:, qof:qof + N])
                        nc.tensor.matmul(sp1[:, k * N:k * N + N],
                                         qkT[:, kof + 128:kof + 256],
                                         qkT[:, qof:qof + N])
                    pump(1)
                    nc.scalar.activation(sT0[:, h0 * N:(h0 + gw) * N],
                                         sp0[:, :gw * N], AF.Identity,
                                         bias=mbt0_sb[:, gi:gi + 1])
                    # tail rows: fold mask into exp, psum -> probs directly
                    nc.scalar.activation(probs1[:N1, h0 * N:(h0 + gw) * N],
                                         sp1[:N1, :gw * N], AF.Exp,
                                         bias=mbt1_sb[:, gi:gi + 1])
                pump(pumps[0])
                # keypoint MLP, all heads batched: [100, 800]; W1/W2 are
                # zero-padded to 128x128 so the MLP matmuls are full-shape too
                aqv = sT0[0:NK, :].rearrange("p (h t) -> p h t", h=H)[:, :, 0:NK]
                raq = raq_bufs[gi % 2]
                raqv = raq[:NK, :].rearrange("p (h t) -> p h t", h=H)
                # drain steps (whole last group): keep the DVE queue short so
                # B's outT evacuations (which gate the reserved final chains)
                # clear early -- the relu/bias epilogues go to ScalarE instead
                drain = gi >= BPC - GB
                if drain:
                    nc.scalar.activation(raqv, aqv, AF.Relu)
                else:
                    nc.vector.tensor_scalar(raqv, aqv, 0.0, None, ALU.max)
                h1 = h1_bufs[gi % 2]
                for c0, cw in ((0, 512), (512, 288)):
                    m1 = pm.tile([128, 512], F32, tag="pm")
                    nc.tensor.matmul(m1[:, :cw], w1_sb[:, :], raq[:, c0:c0 + cw])
                    if drain:
                        nc.scalar.activation(h1[:50, c0:c0 + cw], m1[:50, :cw],
                                             AF.Relu, bias=b1_sb[:])
                    else:
                        nc.vector.tensor_scalar(h1[:50, c0:c0 + cw], m1[:50, :cw],
                                                b1_sb[:], 0.0, ALU.add, ALU.max)
                pump(pumps[1])
                pump_finals(pumps[2])
                lv = mlp_pool.tile([NK, H * NK], BF16, tag="lv")
                for c0, cw in ((0, 512), (512, 288)):
                    m2 = pm.tile([128, 512], F32, tag="pm")
                    nc.tensor.matmul(m2[:, :cw], w2_sb[:, :], h1[:, c0:c0 + cw])
                    if drain:
                        nc.scalar.activation(lv[:, c0:c0 + cw], m2[:NK, :cw],
                                             AF.Relu, bias=b2_sb[:])
                    else:
                        nc.vector.tensor_scalar(lv[:, c0:c0 + cw], m2[:NK, :cw],
                                                b2_sb[:], 0.0, ALU.add, ALU.max)
                # xin = aq + xian * lv  (xian broadcast across heads); the
                # product lands in raq's tile, which is dead after MLP1
                tmpv = raq[:NK, :].rearrange("p (h t) -> p h t", h=H)
                lvv = lv[:, :].rearrange("p (h t) -> p h t", h=H)
                xibv = xian_sb[:, b * NK:(b + 1) * NK].unsqueeze(1).broadcast_to((NK, H, NK))
                nc.vector.tensor_mul(tmpv, lvv, xibv)
                nc.vector.tensor_add(aqv, aqv, tmpv)
                # exp over the full main tile
                nc.scalar.activation(probs0[:], sT0[:], AF.Exp)
                # probs0 + probs1 (tail rows land anywhere -- the denominator
                # matmul sums over partitions) into sT0's now-dead bytes, so
                # the denominator needs one matmul per chunk instead of two.
                # Skipped for the last two batches: the drain is DVE-latency
                # bound there and the two-matmul path has no DVE prologue.
                if gi < BPC - 2:
                    psv = sT0[:, :].bitcast(BF16)[:, :H * N]
                    nc.vector.tensor_add(psv, probs0[:], probs1[:])
                else:
                    psv = None
                state[gi] = (probs0, probs1, psv, outT_p, half, v_sb, b)

            # ---- attention phase B: denominators + AV for batch gi ----
            def emit_B(gi, pumps):
                probs0, probs1, psv, outT_p, half, v_sb, b = state.pop(gi)
                pump(pumps[0])
                rbc = rbc_pool.tile([128, H * N], F32, tag="rbc")
                for c0, cw in ((0, 512), (512, 512), (1024, 288)):
                    dp = pm.tile([128, 512], F32, tag="pm")
                    if psv is not None:
                        nc.tensor.matmul(dp[:, :cw], onesm_sb[:, :],
                                         psv[:, c0:c0 + cw])
                    else:
                        nc.tensor.matmul(dp[:, :cw], onesm_sb[:, :],
                                         probs0[:, c0:c0 + cw], start=True, stop=False)
                        nc.tensor.matmul(dp[:, :cw], onesm_sb[:, :],
                                         probs1[:, c0:c0 + cw], start=False, stop=True)
                    nc.vector.reciprocal_approx_fast(rbc[:, c0:c0 + cw], dp[:, :cw])
                pump(1)
                pump_finals(pumps[1])
                outv = outT_p[:, :H * PW].rearrange("p (h t) -> p h t", h=H)
                for h0, gw in HGRP:
                    oT = psc.tile([128, 492], F32, tag="sc")
                    for k in range(gw):
                        h = h0 + k
                        nc.tensor.matmul(oT[:, k * N:k * N + N],
                                         v_sb[:, (b * 2) * D + h * DH:(b * 2) * D + h * DH + DH],
                                         probs0[:, h * N:h * N + N],
                                         start=True, stop=False)
                        nc.tensor.matmul(oT[:, k * N:k * N + N],
                                         v_sb[:, (b * 2 + 1) * D + h * DH:(b * 2 + 1) * D + h * DH + DH],
                                         probs1[:, h * N:h * N + N],
                                         start=False, stop=True)
                    pump(1)
                    nc.vector.tensor_mul(
                        outv[:, h0:h0 + gw, half * N:half * N + N],
                        oT[:, :gw * N].rearrange("p (h t) -> p h t", h=gw),
                        rbc[:, h0 * N:(h0 + gw) * N].rearrange("p (h t) -> p h t", h=gw),
                    )
                p = gi // 2
                if half == 0:
                    # i-tile 0 covers only this (even) batch's token columns
                    finals.append(final_chain(outT_p, p, 0))
                else:
                    finals.append(final_chain(outT_p, p, 1))
                    finals.append(final_chain(outT_p, p, 2))

            # ================= main schedule =================
            # prologue: group 0 inputs, weights, group 0 chains; spread the
            # startup loads across engine DMA queues so they run in parallel
            g0_chains, _ = start_group(0)
            for c in range(16):
                nc.gpsimd.dma_start(wqk_sb[:, c * 1024:(c + 1) * 1024],
                                    d_wqk.ap()[:, c * 1024:(c + 1) * 1024])
            load_consts_early()
            load_consts_late()
            # warm the PE (p-state ramp) while the first DMAs land; onesm is
            # produced by an on-chip memset so this depends on no DMA
            wup = pp.tile([128, 512], F32, tag="proj")
            for i in range(64):
                nc.tensor.matmul(wup[:, :128], onesm_sb[:, :], onesm_sb[:, :])
            for gen in g0_chains:
                for _ in gen:
                    pass

            last_pair1 = None
            for gi in range(BPC):
                g, b = divmod(gi, GB)
                if b == 0:
                    if g + 1 < NG:
                        chains, tail = start_group(g + 1)
                        work.extend(chains)
                        if tail is not None:
                            last_pair1 = tail
                    elif last_pair1 is not None:
                        work.extend(last_pair1)
                        last_pair1 = None
                # pacing: spread remaining queued chain units (1 unit = 1 MM)
                # over the remaining steps of this group window (last group:
                # pair1 chains must land within its first two steps)
                if g < NG - 1:
                    window_left = GB - b
                else:
                    window_left = max(1, 2 - b)
                quota = -(-work_units() // window_left)
                # ~7 chains are consumed by the fine-grained pumps inside
                # emit_A/emit_B; boundary pumps soak up the rest
                qb = max(0, (quota - 6) // 2)
                # reserve the last pairs' final chains to cover the drain; in
                # the drain region emit B first so its recip/outT evacuations
                # queue on DVE ahead of A's MLP chain (finals unblock sooner)
                fa = 0 if gi >= BPC - 2 else 2
                emit_A(gi, pumps=(qb, qb, fa))
                if gi > 0:
                    emit_B(gi - 1, pumps=(1, fa))
                if b == (GB - 1 if g < NG - 1 else 1):
                    pump_work_all()

            emit_B(BPC - 1, pumps=(2, 2))
            while finals or work:
                pump(99)
                pump_finals(99)

    nc.compile()
    return nc


def _prep_core_inputs(xc, maskc, xianc, shared):
    # xT tiles: [BPC,N,D] -> (g, p, dt, b, n) -> [NG, 128, 8*GB*N]
    xt = xc.transpose(0, 2, 1).reshape(NG, GB, 8, 128, N)
    xt = np.ascontiguousarray(xt.transpose(0, 3, 2, 1, 4)).reshape(NG, 128, 8 * XCOLS)
    xt = xt.astype(nbf16)
    # mask bias transposed: [164, BPC]
    mb = np.where(maskc, np.float32(MASK_NEG), np.float32(0.0)).astype(np.float32)
    mbt = np.ascontiguousarray(mb.T)
    # xianT: [BPC,100,100] -> xianT[b][j,i] = xian[b][i,j] -> (g, j, b, i)
    xiant = xianc.transpose(0, 2, 1).reshape(NG, GB, NK, NK)
    xiant = np.ascontiguousarray(xiant.transpose(0, 2, 1, 3)).reshape(NG, NK, GB * NK)
    xiant = xiant.astype(nbf16)
    m = {
        "xt": xt,
        "mbt0": np.ascontiguousarray(mbt[:128]),
        "mbt1": np.ascontiguousarray(mbt[128:]),
        "xiant": xiant,
    }
    m.update(shared)
    return m


def kernel(x, mask, xian, Wqkv, W1, b1, W2, b2, Wout, bout):
    global LAST_EXEC_NS
    x = np.asarray(x, dtype=np.float32)
    mask = np.asarray(mask)
    xian = np.asarray(xian, dtype=np.float32)
    Wqkv = np.asarray(Wqkv, dtype=np.float32)
    W1 = np.asarray(W1, dtype=np.float32)
    b1 = np.asarray(b1, dtype=np.float32)
    W2 = np.asarray(W2, dtype=np.float32)
    b2 = np.asarray(b2, dtype=np.float32)
    Wout = np.asarray(Wout, dtype=np.float32)
    bout = np.asarray(bout, dtype=np.float32)

    if "nc" not in _CACHE:
        _CACHE["nc"] = _build_nc()
    nc = _CACHE["nc"]

    # ---- shared weight prep (scale folded into Wq) ----
    scale = np.float32(D ** -0.5)
    wqkv_s = Wqkv.copy()
    wqkv_s[:, :D] *= scale
    # wqk ct-major: [dt,p, ct,c] -> [p, ct, dt, c]
    wqk = wqkv_s[:, :2 * D].reshape(8, 128, 16, 128)
    wqk_h = np.ascontiguousarray(wqk.transpose(1, 2, 0, 3)).reshape(128, 16 * 1024).astype(nbf16)
    # wv dt-major: [dt, p, c] -> [p, dt, c]
    wv = wqkv_s[:, 2 * D:].reshape(8, 128, 1024)
    wv_h = np.ascontiguousarray(wv.transpose(1, 0, 2)).reshape(128, 8 * 1024).astype(nbf16)
    wout_h = np.ascontiguousarray(
        Wout.reshape(8, 128, D).transpose(1, 0, 2)).reshape(128, 8 * D).astype(nbf16)
    shared = {
        "wqk": wqk_h,
        "wv": wv_h,
        "wout": wout_h,
        "w1": W1.astype(nbf16),
        "w2": W2.astype(nbf16),
        "b1c": np.ascontiguousarray(b1.reshape(50, 1)),
        "b2c": np.ascontiguousarray(b2.reshape(NK, 1)),
        "boutb": np.ascontiguousarray(np.broadcast_to(bout, (128, D))).astype(nbf16),
    }

    in_maps = []
    for c in range(NCORES):
        sl = slice(c * BPC, (c + 1) * BPC)
        in_maps.append(_prep_core_inputs(x[sl], mask[sl], xian[sl], shared))

    trace = bool(int(os.environ.get("KERNEL_TRACE", "0")))
    if trace:
        trace = _install_profile_hook()
    res = bass_utils.run_bass_kernel_spmd(
        nc, in_maps, core_ids=list(range(NCORES)), trace=trace)
    LAST_EXEC_NS = res.exec_time_ns

    out = np.empty((B, N, D), dtype=np.float32)
    for c in range(NCORES):
        out[c * BPC:(c + 1) * BPC] = res.results[c]["y"].reshape(BPC, N, D).astype(np.float32)
    return out


# revision 76
# speedup vs baseline: 1.0104x; 1.0104x over previous
"""Trainium2 Bass kernel for nn_Attention_33921651703853 (sparse_attention).

Data-parallel over batch: B=256 -> 32 batches on each of 8 NeuronCores.
All weights replicated; no collectives.

Design (v4 — software-pipelined, head-batched, uniform-shape matmuls):
  - Everything transposed on device (no on-device transposes):
    xT tiles [d-part, token-free] bf16, host-pretiled; scores computed as
    sT[j, i]; AV gives outT[d, i]; final projection consumes outT directly.
  - EVERY matmul is padded to a K=128 x M=128 PE-array shape: switching the
    array tile config between matmuls flushes the PE pipeline (measured
    3.2x slowdown on alternating-shape streams). Token/row tails (36 of
    164) read past their logical end into small zero/garbage extensions;
    K-padding is exact because probs1/v/raq/h1 carry zeroed pad rows.
  - Attention is processed per batch with all 8 heads batched:
    scores for head-groups (3,3,2) land in shared PSUM tiles and are
    evacuated in one activation per group (mask folded as per-partition
    bias). The keypoint-block MLP runs once per batch over [100, 8*100]
    with W1/W2 zero-padded to 128x128, its epilogues are single DVE ops,
    xian is broadcast across heads with a stride-0 AP. exp runs as one
    activation over [128, 8*164]; the j-tail goes psum->exp directly with
    the mask bias folded into Exp (no SBUF staging).
  - Softmax denominators: probs0+probs1 are pre-added on DVE into the dead
    sT0 tile reinterpreted as bf16 (partition sums are row-order agnostic),
    so one ones-matmul per 512-col chunk computes all 8 heads' denominators
    broadcast across partitions. Reciprocal on DVE; AV results are
    normalized during the PSUM->SBUF evacuation multiply per head group.
  - Batches are software-pipelined: step i emits phase A(i) (scores+MLP+exp)
    interleaved with phase B(i-1) (denom+AV) of the previous batch, so no
    engine waits on a freshly produced dependency.
  - Final projection is packed across batch PAIRS: outT pair tiles
    [128, 8h*328] give i-tiles (128,128,72+pad) instead of (128,36)x2 ->
    25% fewer streamed PE columns. y rows are contiguous global tokens.
  - QKV weights are stored ct-major (wqk) and streamed in 16 chunks on the
    gpsimd DMA queue (xt on the sync queue in parallel); dummy warmup
    matmuls on onesm ramp the PE p-state while the first DMAs land.
    Projection/final chains of the next group are zipped into the current
    group's attention stream with deadline pacing, and the last pairs'
    final chains are held back to cover the pipeline drain.
  Result: TensorE ~97% busy at ~0.44 ns/col (HFU ~0.90); 1376us -> 830us.
"""

import os
from collections import deque
import numpy as np
import ml_dtypes

import concourse.bass as bass
import concourse.bacc as bacc
import concourse.mybir as mybir
import concourse.tile as tile
from concourse import bass_utils

BF16 = mybir.dt.bfloat16
F32 = mybir.dt.float32
AF = mybir.ActivationFunctionType
ALU = mybir.AluOpType
nbf16 = ml_dtypes.bfloat16

B, N, D, H, NK, DH = 256, 164, 1024, 8, 100, 128
NCORES = 8
BPC = B // NCORES          # 32 batches per core
GB = 4                     # batches per group
NG = BPC // GB             # 8 groups
XCOLS = GB * N             # 656 free cols per d-tile in xT sbuf
PW = 2 * N                 # 328 pair width (final projection packing)
N1 = N - 128               # 36 tail rows
MASK_NEG = -1.0e30
HGRP = [(0, 3), (3, 3), (6, 2)]   # head groups for score/AV psum batching
# Every matmul is padded to a K=128 x M=128 PE-array shape: switching the
# array tile config between matmuls flushes the PE pipeline (measured 3.2x
# slowdown on alternating-shape streams). Stationary operands read up to 92
# columns past their logical end, so the staging tiles carry a small
# zero-initialized extension.
EXT = 92
QKT_W = 16 * XCOLS + EXT
XT_W = 8 * XCOLS + EXT
OUTT_W = H * PW + 56

_CACHE = {}
LAST_EXEC_NS = None


def _install_profile_hook():
    """Make run_bass_kernel_spmd(trace=True) work under axon in this image."""
    import sys as _sys
    import types as _types
    try:
        import antenv  # noqa: F401
        try:
            from antenv.axon_hooks import get_axon_ntff_profile_hook  # noqa: F401
        except ImportError:
            from trn_agent_boot.trn_boot import _ntff_profile_via_ctypes
            hook = _ntff_profile_via_ctypes("/opt/axon/libaxon_pjrt.so")
            mod = _types.ModuleType("antenv.axon_hooks")
            mod._hook = hook
            mod.set_axon_ntff_profile_hook = lambda h: setattr(mod, "_hook", h)
            mod.get_axon_ntff_profile_hook = lambda: mod._hook
            _sys.modules["antenv.axon_hooks"] = mod
            antenv.axon_hooks = mod
        if not getattr(bass_utils, "_upload_patched", False):
            _orig_upload = bass_utils.upload_artifacts

            def _safe_upload(tmpdir):
                try:
                    return _orig_upload(tmpdir)
                except Exception:
                    return tmpdir

            bass_utils.upload_artifacts = _safe_upload
            bass_utils._upload_patched = True
        return True
    except Exception as e:  # pragma: no cover
        print(f"profile hook install failed: {type(e).__name__}: {e}")
        return False


def _build_nc():
    nc = bacc.Bacc("TRN2", target_bir_lowering=False, debug=False)

    # ---- DRAM parameters (per-core shapes) ----
    d_xt = nc.dram_tensor("xt", [NG, 128, 8 * XCOLS], BF16, kind="ExternalInput")
    d_wqk = nc.dram_tensor("wqk", [128, 16 * 1024], BF16, kind="ExternalInput")
    d_wv = nc.dram_tensor("wv", [128, 8 * 1024], BF16, kind="ExternalInput")
    d_wout = nc.dram_tensor("wout", [128, 8 * D], BF16, kind="ExternalInput")
    d_w1 = nc.dram_tensor("w1", [NK, 50], BF16, kind="ExternalInput")
    d_w2 = nc.dram_tensor("w2", [50, NK], BF16, kind="ExternalInput")
    d_b1 = nc.dram_tensor("b1c", [50, 1], F32, kind="ExternalInput")
    d_b2 = nc.dram_tensor("b2c", [NK, 1], F32, kind="ExternalInput")
    d_boutb = nc.dram_tensor("boutb", [128, D], BF16, kind="ExternalInput")
    d_mbt0 = nc.dram_tensor("mbt0", [128, BPC], F32, kind="ExternalInput")
    d_mbt1 = nc.dram_tensor("mbt1", [N1, BPC], F32, kind="ExternalInput")
    d_xiant = nc.dram_tensor("xiant", [NG, NK, GB * NK], BF16, kind="ExternalInput")
    d_y = nc.dram_tensor("y", [BPC * N, D], BF16, kind="ExternalOutput")

    xt_ap = d_xt.ap()
    y_ap = d_y.ap()
    xiant_ap = d_xiant.ap()

    with tile.TileContext(nc) as tc:
        with (
            tc.tile_pool(name="const", bufs=1) as cpool,
            tc.tile_pool(name="xt", bufs=2) as xt_pool,
            tc.tile_pool(name="xian", bufs=2) as xian_pool,
            tc.tile_pool(name="qk", bufs=2) as qk_pool,
            tc.tile_pool(name="st0", bufs=2) as st0_pool,
            tc.tile_pool(name="pr0", bufs=1) as pr0_pool,
            tc.tile_pool(name="mlp", bufs=1) as mlp_pool,
            tc.tile_pool(name="rbcp", bufs=1) as rbc_pool,
            tc.tile_pool(name="outT", bufs=2) as outT_pool,
            tc.tile_pool(name="ysb", bufs=3) as y_pool,
            tc.tile_pool(name="pproj", bufs=3, space="PSUM") as pp,
            tc.tile_pool(name="psc", bufs=3, space="PSUM") as psc,
            tc.tile_pool(name="pmx", bufs=2, space="PSUM") as pm,
        ):
            # ---- constant tiles ----
            wqk_sb = cpool.tile([128, 16 * 1024], BF16, tag="wqk")
            wv_sb = cpool.tile([128, 8 * 1024], BF16, tag="wv")
            wout_sb = cpool.tile([128, 8 * D], BF16, tag="wout")
            w1_sb = cpool.tile([128, 128], BF16, tag="w1")
            w2_sb = cpool.tile([128, 128], BF16, tag="w2")
            b1_sb = cpool.tile([50, 1], F32, tag="b1")
            b2_sb = cpool.tile([NK, 1], F32, tag="b2")
            boutb_sb = cpool.tile([128, D], BF16, tag="boutb")
            mbt0_sb = cpool.tile([128, BPC], F32, tag="mbt0")
            mbt1_sb = cpool.tile([N1, BPC], F32, tag="mbt1")
            onesm_sb = cpool.tile([128, 128], BF16, tag="onesm")
            nc.vector.memset(onesm_sb[:], 1.0)
            # persistent double buffers with zero-padded K rows (so padded
            # K=128 matmuls contribute exactly zero from rows past the data)
            # memset bases must be 32-aligned; rows below the logical pad line
            # are rewritten by the regular per-batch writes before any read
            def zero_rows(t):
                # partition spans >32 must start at 0 or 64
                nc.vector.memset(t[32:64, :], 0.0)
                nc.vector.memset(t[64:128, :], 0.0)

            v_bufs = []
            for vi in range(2):
                vt = cpool.tile([128, GB * 2 * D], BF16, tag=f"v{vi}", name=f"v{vi}")
                zero_rows(vt)
                v_bufs.append(vt)
            p1_bufs = []
            for vi in range(2):
                p1 = cpool.tile([128, H * N], BF16, tag=f"p1{vi}", name=f"p1{vi}")
                zero_rows(p1)
                p1_bufs.append(p1)
            raq_bufs = []
            for vi in range(2):
                rq = cpool.tile([128, H * NK], BF16, tag=f"rq{vi}", name=f"rq{vi}")
                nc.vector.memset(rq[96:128, :], 0.0)
                raq_bufs.append(rq)
            h1_bufs = []
            for vi in range(2):
                hh = cpool.tile([128, H * NK], BF16, tag=f"h{vi}", name=f"h{vi}")
                zero_rows(hh)
                h1_bufs.append(hh)
            nc.vector.memset(w1_sb[:], 0.0)
            nc.vector.memset(w2_sb[:], 0.0)

            def load_consts_early():
                # small tiles needed by the first attention batch
                nc.sync.dma_start(w1_sb[:NK, :50], d_w1.ap()[:, :])
                nc.sync.dma_start(w2_sb[:50, :NK], d_w2.ap()[:, :])
                nc.sync.dma_start(b1_sb[:], d_b1.ap()[:, :])
                nc.sync.dma_start(b2_sb[:], d_b2.ap()[:, :])
                nc.sync.dma_start(mbt0_sb[:], d_mbt0.ap()[:, :])
                nc.sync.dma_start(mbt1_sb[:], d_mbt1.ap()[:, :])

            def load_consts_late():
                nc.gpsimd.dma_start(wv_sb[:, :4096], d_wv.ap()[:, :4096])
                nc.gpsimd.dma_start(wv_sb[:, 4096:], d_wv.ap()[:, 4096:])
                nc.gpsimd.dma_start(wout_sb[:, :4096], d_wout.ap()[:, :4096])
                nc.gpsimd.dma_start(wout_sb[:, 4096:], d_wout.ap()[:, 4096:])
                nc.gpsimd.dma_start(boutb_sb[:], d_boutb.ap()[:, :])

            # ---- projection chains ----
            group_tiles = {}

            def start_group(g):
                """Issue group g's input DMAs, allocate tiles, return the list
                of projection-chain closures (qk then v)."""
                xt_sb = xt_pool.tile([128, XT_W], BF16, tag="xt")
                nc.sync.dma_start(xt_sb[:, :8 * XCOLS], xt_ap[g, :, :])
                nc.vector.memset(xt_sb[:, 8 * XCOLS:], 0.0)
                xian_sb = xian_pool.tile([NK, GB * NK], BF16, tag="xian")
                nc.sync.dma_start(xian_sb[:], xiant_ap[g, :, :])
                qkT = qk_pool.tile([128, QKT_W], BF16, tag="qkT")
                nc.vector.memset(qkT[:, 16 * XCOLS:], 0.0)
                v_sb = v_bufs[g % 2]
                group_tiles[g] = (xt_sb, qkT, v_sb, xian_sb)

                def qk_chain(ct, c0, cw):
                    pt = pp.tile([128, 512], F32, tag="proj")
                    for dt in range(8):
                        nc.tensor.matmul(
                            pt[:, :cw],
                            wqk_sb[:, ct * 1024 + dt * 128:ct * 1024 + dt * 128 + 128],
                            xt_sb[:, dt * XCOLS + c0:dt * XCOLS + c0 + cw],
                            start=(dt == 0), stop=(dt == 7),
                        )
                        if dt < 7:
                            yield
                    nc.scalar.activation(
                        qkT[:, ct * XCOLS + c0:ct * XCOLS + c0 + cw],
                        pt[:, :cw], AF.Copy,
                    )

                def v_chain(b, tt, ch):
                    # stationary always 128 tokens; the tt=1 tile reads 92
                    # cols past the batch tail (next batch / zero extension),
                    # producing dead psum rows 36:128 that are never evacuated
                    p0, pw = (0, 128) if tt == 0 else (128, N1)
                    pt = pp.tile([128, 512], F32, tag="proj")
                    for dt in range(8):
                        nc.tensor.matmul(
                            pt[:, :],
                            xt_sb[:, dt * XCOLS + b * N + p0:dt * XCOLS + b * N + p0 + 128],
                            wv_sb[:, dt * 1024 + ch * 512:dt * 1024 + ch * 512 + 512],
                            start=(dt == 0), stop=(dt == 7),
                        )
                        if dt < 7:
                            yield
                    nc.vector.tensor_copy(
                        v_sb[:pw, (b * 2 + tt) * D + ch * 512:(b * 2 + tt) * D + ch * 512 + 512],
                        pt[:pw, :],
                    )

                if g < NG - 1:
                    chains = []
                    for ct in range(16):
                        for c0, cw in ((0, 512), (512, XCOLS - 512)):
                            chains.append(qk_chain(ct, c0, cw))
                    for b in range(GB):
                        for tt in range(2):
                            for ch in range(2):
                                chains.append(v_chain(b, tt, ch))
                    return chains, None
                # last group: split per pair; pair chains must finish before
                # that pair's attention
                pairs = []
                for p in range(2):
                    L = []
                    for ct in range(16):
                        L.append(qk_chain(ct, p * PW, PW))
                    for b in (2 * p, 2 * p + 1):
                        for tt in range(2):
                            for ch in range(2):
                                L.append(v_chain(b, tt, ch))
                    pairs.append(L)
                return pairs[0], pairs[1]

            # ---- final projection (packed per batch pair) ----
            def final_chain(outT_p, p, it):
                i0 = it * 128
                iw = 128 if it < 2 else PW - 256
                for ch in range(2):
                    y_sb = y_pool.tile([128, 512], BF16, tag="y")
                    yp = pp.tile([128, 512], F32, tag="proj")
                    for h2 in range(H):
                        nc.tensor.matmul(
                            yp[:, :],
                            outT_p[:, h2 * PW + i0:h2 * PW + i0 + 128],
                            wout_sb[:, h2 * D + ch * 512:h2 * D + ch * 512 + 512],
                            start=(h2 == 0), stop=(h2 == 7),
                        )
                        if h2 < 7:
                            yield
                    nc.vector.tensor_add(
                        y_sb[:iw, :],
                        yp[:iw, :],
                        boutb_sb[:iw, ch * 512:ch * 512 + 512],
                    )
                    nc.sync.dma_start(
                        y_ap[p * PW + i0:p * PW + i0 + iw, ch * 512:ch * 512 + 512],
                        y_sb[:iw, :])

            # ---- work queues & pacing (generator granularity: 1 unit = 1 MM) ----
            work = deque()     # projection chain generators (window deadlines)
            finals = deque()   # final-chain generators (soft deadlines)
            cur = {"w": None, "f": None}

            def _run(gen):
                for _ in gen:
                    pass

            def pump(n):
                # n counts whole chains; chains have window deadlines
                while n > 0 and (work or finals):
                    q = work if work else finals
                    _run(q.popleft())
                    n -= 1

            def pump_finals(n):
                while n > 0 and finals:
                    _run(finals.popleft())
                    n -= 1

            def pump_work_all():
                while work:
                    _run(work.popleft())

            def work_units():
                return len(work)

            # ---- attention phase A: scores + MLP + exp for batch gi ----
            state = {}
            pair_outT = {}

            def emit_A(gi, pumps):
                g, b = divmod(gi, GB)
                xt_sb, qkT, v_sb, xian_sb = group_tiles[g]
                half = b & 1
                if half == 0:
                    outT_p = outT_pool.tile([128, OUTT_W], BF16, tag="outT")
                    nc.vector.memset(outT_p[:, H * PW:], 0.0)
                    pair_outT[gi // 2] = outT_p
                else:
                    outT_p = pair_outT[gi // 2]
                sT0 = st0_pool.tile([128, H * N], F32, tag="sT0")
                probs0 = pr0_pool.tile([128, H * N], BF16, tag="p0")
                probs1 = p1_bufs[gi % 2]
                # scores per head group; all matmuls K=128 x M=128 (the k-tail
                # stationary reads 92 cols past the batch: dead psum rows)
                for h0, gw in HGRP:
                    sp0 = psc.tile([128, 492], F32, tag="sc")
                    sp1 = pm.tile([128, 512], F32, tag="pm")
                    for k in range(gw):
                        h = h0 + k
                        qof = h * XCOLS + b * N
                        kof = (8 + h) * XCOLS + b * N
                        nc.tensor.matmul(sp0[:, k * N:k * N + N],
                                         qkT[:, kof:kof + 128],
                                         qkT[:, qof:qof + N])
                        nc.tensor.matmul(sp1[:, k * N:k * N + N],
                                         qkT[:, kof + 128:kof + 256],
                                         qkT[:, qof:qof + N])
                    pump(1)
                    nc.scalar.activation(sT0[:, h0 * N:(h0 + gw) * N],
                                         sp0[:, :gw * N], AF.Identity,
                                         bias=mbt0_sb[:, gi:gi + 1])
                    # tail rows: fold mask into exp, psum -> probs directly
                    nc.scalar.activation(probs1[:N1, h0 * N:(h0 + gw) * N],
                                         sp1[:N1, :gw * N], AF.Exp,
                                         bias=mbt1_sb[:, gi:gi + 1])
                pump(pumps[0])
                # keypoint MLP, all heads batched: [100, 800]; W1/W2 are
                # zero-padded to 128x128 so the MLP matmuls are full-shape too
                aqv = sT0[0:NK, :].rearrange("p (h t) -> p h t", h=H)[:, :, 0:NK]
                raq = raq_bufs[gi % 2]
                raqv = raq[:NK, :].rearrange("p (h t) -> p h t", h=H)
                # drain steps (whole last group): keep the DVE queue short so
                # B's outT evacuations (which gate the reserved final chains)
                # clear early -- the relu/bias epilogues go to ScalarE instead
                drain = gi >= BPC - GB
                if drain:
                    nc.scalar.activation(raqv, aqv, AF.Relu)
                else:
                    nc.vector.tensor_scalar(raqv, aqv, 0.0, None, ALU.max)
                h1 = h1_bufs[gi % 2]
                for c0, cw in ((0, 512), (512, 288)):
                    m1 = pm.tile([128, 512], F32, tag="pm")
                    nc.tensor.matmul(m1[:, :cw], w1_sb[:, :], raq[:, c0:c0 + cw])
                    if drain:
                        nc.scalar.activation(h1[:50, c0:c0 + cw], m1[:50, :cw],
                                             AF.Relu, bias=b1_sb[:])
                    else:
                        nc.vector.tensor_scalar(h1[:50, c0:c0 + cw], m1[:50, :cw],
                                                b1_sb[:], 0.0, ALU.add, ALU.max)
                pump(pumps[1])
                pump_finals(pumps[2])
                lv = mlp_pool.tile([NK, H * NK], BF16, tag="lv")
                for c0, cw in ((0, 512), (512, 288)):
                    m2 = pm.tile([128, 512], F32, tag="pm")
                    nc.tensor.matmul(m2[:, :cw], w2_sb[:, :], h1[:, c0:c0 + cw])
                    if drain:
                        nc.scalar.activation(lv[:, c0:c0 + cw], m2[:NK, :cw],
                                             AF.Relu, bias=b2_sb[:])
                    else:
                        nc.vector.tensor_scalar(lv[:, c0:c0 + cw], m2[:NK, :cw],
                                                b2_sb[:], 0.0, ALU.add, ALU.max)
                # xin = aq + xian * lv  (xian broadcast across heads); the
                # product lands in raq's tile, which is dead after MLP1
                tmpv = raq[:NK, :].rearrange("p (h t) -> p h t", h=H)
                lvv = lv[:, :].rearrange("p (h t) -> p h t", h=H)
                xibv = xian_sb[:, b * NK:(b + 1) * NK].unsqueeze(1).broadcast_to((NK, H, NK))
                nc.vector.tensor_mul(tmpv, lvv, xibv)
                nc.vector.tensor_add(aqv, aqv, tmpv)
                # exp over the full main tile
                nc.scalar.activation(probs0[:], sT0[:], AF.Exp)
                # probs0 + probs1 (tail rows land anywhere -- the denominator
                # matmul sums over partitions) into sT0's now-dead bytes, so
                # the denominator needs one matmul per chunk instead of two.
                # Skipped for the last two batches: the drain is DVE-latency
                # bound there and the two-matmul path has no DVE prologue.
                if gi < BPC - 2:
                    psv = sT0[:, :].bitcast(BF16)[:, :H * N]
                    nc.vector.tensor_add(psv, probs0[:], probs1[:])
                else:
                    psv = None
                state[gi] = (probs0, probs1, psv, outT_p, half, v_sb, b)

            # ---- attention phase B: denominators + AV for batch gi ----
            def emit_B(gi, pumps):
                probs0, probs1, psv, outT_p, half, v_sb, b = state.pop(gi)
                pump(pumps[0])
                rbc = rbc_pool.tile([128, H * N], F32, tag="rbc")
                for c0, cw in ((0, 512), (512, 512), (1024, 288)):
                    dp = pm.tile([128, 512], F32, tag="pm")
                    if psv is not None:
                        nc.tensor.matmul(dp[:, :cw], onesm_sb[:, :],
                                         psv[:, c0:c0 + cw])
                    else:
                        nc.tensor.matmul(dp[:, :cw], onesm_sb[:, :],
                                         probs0[:, c0:c0 + cw], start=True, stop=False)
                        nc.tensor.matmul(dp[:, :cw], onesm_sb[:, :],
                                         probs1[:, c0:c0 + cw], start=False, stop=True)
                    nc.vector.reciprocal_approx_fast(rbc[:, c0:c0 + cw], dp[:, :cw])
                pump(1)
                pump_finals(pumps[1])
                outv = outT_p[:, :H * PW].rearrange("p (h t) -> p h t", h=H)
                for h0, gw in HGRP:
                    oT = psc.tile([128, 492], F32, tag="sc")
                    for k in range(gw):
                        h = h0 + k
                        nc.tensor.matmul(oT[:, k * N:k * N + N],
                                         v_sb[:, (b * 2) * D + h * DH:(b * 2) * D + h * DH + DH],
                                         probs0[:, h * N:h * N + N],
                                         start=True, stop=False)
                        nc.tensor.matmul(oT[:, k * N:k * N + N],
                                         v_sb[:, (b * 2 + 1) * D + h * DH:(b * 2 + 1) * D + h * DH + DH],
                                         probs1[:, h * N:h * N + N],
                                         start=False, stop=True)
                    pump(1)
                    nc.vector.tensor_mul(
                        outv[:, h0:h0 + gw, half * N:half * N + N],
                        oT[:, :gw * N].rearrange("p (h t) -> p h t", h=gw),
                        rbc[:, h0 * N:(h0 + gw) * N].rearrange("p (h t) -> p h t", h=gw),
                    )
                p = gi // 2
                if half == 0:
                    # i-tile 0 covers only this (even) batch's token columns
                    finals.append(final_chain(outT_p, p, 0))
                else:
                    finals.append(final_chain(outT_p, p, 1))
                    finals.append(final_chain(outT_p, p, 2))

            # ================= main schedule =================
            # prologue: group 0 inputs, weights, group 0 chains; spread the
            # startup loads across engine DMA queues so they run in parallel
            g0_chains, _ = start_group(0)
            for c in range(16):
                nc.gpsimd.dma_start(wqk_sb[:, c * 1024:(c + 1) * 1024],
                                    d_wqk.ap()[:, c * 1024:(c + 1) * 1024])
            load_consts_early()
            load_consts_late()
            # warm the PE (p-state ramp) while the first DMAs land; onesm is
            # produced by an on-chip memset so this depends on no DMA
            wup = pp.tile([128, 512], F32, tag="proj")
            for i in range(64):
                nc.tensor.matmul(wup[:, :128], onesm_sb[:, :], onesm_sb[:, :])
            for gen in g0_chains:
                for _ in gen:
                    pass

            last_pair1 = None
            for gi in range(BPC):
                g, b = divmod(gi, GB)
                if b == 0:
                    if g + 1 < NG:
                        chains, tail = start_group(g + 1)
                        work.extend(chains)
                        if tail is not None:
                            last_pair1 = tail
                    elif last_pair1 is not None:
                        work.extend(last_pair1)
                        last_pair1 = None
                # pacing: spread remaining queued chain units (1 unit = 1 MM)
                # over the remaining steps of this group window (last group:
                # pair1 chains must land within its first two steps)
                if g < NG - 1:
                    window_left = GB - b
                else:
                    window_left = max(1, 2 - b)
                quota = -(-work_units() // window_left)
                # ~7 chains are consumed by the fine-grained pumps inside
                # emit_A/emit_B; boundary pumps soak up the rest
                qb = max(0, (quota - 6) // 2)
                # reserve the last pairs' final chains to cover the drain; in
                # the drain region emit B first so its recip/outT evacuations
                # queue on DVE ahead of A's MLP chain (finals unblock sooner)
                fa = 0 if gi >= BPC - 2 else 2
                emit_A(gi, pumps=(qb, qb, fa))
                if gi > 0:
                    emit_B(gi - 1, pumps=(1, fa))
                if b == (GB - 1 if g < NG - 1 else 1):
                    pump_work_all()

            emit_B(BPC - 1, pumps=(2, 2))
            while finals or work:
                pump(99)
                pump_finals(99)

    nc.compile()
    return nc


def _prep_core_inputs(xc, maskc, xianc, shared):
    # xT tiles: [BPC,N,D] -> (g, p, dt, b, n) -> [NG, 128, 8*GB*N]
    xt = xc.transpose(0, 2, 1).reshape(NG, GB, 8, 128, N)
    xt = np.ascontiguousarray(xt.transpose(0, 3, 2, 1, 4)).reshape(NG, 128, 8 * XCOLS)
    xt = xt.astype(nbf16)
    # mask bias transposed: [164, BPC]
    mb = np.where(maskc, np.float32(MASK_NEG), np.float32(0.0)).astype(np.float32)
    mbt = np.ascontiguousarray(mb.T)
    # xianT: [BPC,100,100] -> xianT[b][j,i] = xian[b][i,j] -> (g, j, b, i)
    xiant = xianc.transpose(0, 2, 1).reshape(NG, GB, NK, NK)
    xiant = np.ascontiguousarray(xiant.transpose(0, 2, 1, 3)).reshape(NG, NK, GB * NK)
    xiant = xiant.astype(nbf16)
    m = {
        "xt": xt,
        "mbt0": np.ascontiguousarray(mbt[:128]),
        "mbt1": np.ascontiguousarray(mbt[128:]),
        "xiant": xiant,
    }
    m.update(shared)
    return m


def kernel(x, mask, xian, Wqkv, W1, b1, W2, b2, Wout, bout):
    global LAST_EXEC_NS
    x = np.asarray(x, dtype=np.float32)
    mask = np.asarray(mask)
    xian = np.asarray(xian, dtype=np.float32)
    Wqkv = np.asarray(Wqkv, dtype=np.float32)
    W1 = np.asarray(W1, dtype=np.float32)
    b1 = np.asarray(b1, dtype=np.float32)
    W2 = np.asarray(W2, dtype=np.float32)
    b2 = np.asarray(b2, dtype=np.float32)
    Wout = np.asarray(Wout, dtype=np.float32)
    bout = np.asarray(bout, dtype=np.float32)

    if "nc" not in _CACHE:
        _CACHE["nc"] = _build_nc()
    nc = _CACHE["nc"]

    # ---- shared weight prep (scale folded into Wq) ----
    scale = np.float32(D ** -0.5)
    wqkv_s = Wqkv.copy()
    wqkv_s[:, :D] *= scale
    # wqk ct-major: [dt,p, ct,c] -> [p, ct, dt, c]
    wqk = wqkv_s[:, :2 * D].reshape(8, 128, 16, 128)
    wqk_h = np.ascontiguousarray(wqk.transpose(1, 2, 0, 3)).reshape(128, 16 * 1024).astype(nbf16)
    # wv dt-major: [dt, p, c] -> [p, dt, c]
    wv = wqkv_s[:, 2 * D:].reshape(8, 128, 1024)
    wv_h = np.ascontiguousarray(wv.transpose(1, 0, 2)).reshape(128, 8 * 1024).astype(nbf16)
    wout_h = np.ascontiguousarray(
        Wout.reshape(8, 128, D).transpose(1, 0, 2)).reshape(128, 8 * D).astype(nbf16)
    shared = {
        "wqk": wqk_h,
        "wv": wv_h,
        "wout": wout_h,
        "w1": W1.astype(nbf16),
        "w2": W2.astype(nbf16),
        "b1c": np.ascontiguousarray(b1.reshape(50, 1)),
        "b2c": np.ascontiguousarray(b2.reshape(NK, 1)),
        "boutb": np.ascontiguousarray(np.broadcast_to(bout, (128, D))).astype(nbf16),
    }

    in_maps = []
    for c in range(NCORES):
        sl = slice(c * BPC, (c + 1) * BPC)
        in_maps.append(_prep_core_inputs(x[sl], mask[sl], xian[sl], shared))

    trace = bool(int(os.environ.get("KERNEL_TRACE", "0")))
    if trace:
        trace = _install_profile_hook()
    res = bass_utils.run_bass_kernel_spmd(
        nc, in_maps, core_ids=list(range(NCORES)), trace=trace)
    LAST_EXEC_NS = res.exec_time_ns

    out = np.empty((B, N, D), dtype=np.float32)
    for c in range(NCORES):
        out[c * BPC:(c + 1) * BPC] = res.results[c]["y"].reshape(BPC, N, D).astype(np.float32)
    return out
